# revision 5
# baseline (speedup 1.0000x reference)
"""Trainium2 Bass kernel for nn_MultiHeadedAttention_4269197492266.

Dual-branch multi-head attention where the "local" key path is a multi-scale
conv (k=3,5) + batchnorm + projection.  Host-side algebra folds the whole
local path into a single 5-tap convolution:

    kl = bn(concat(conv3(key), conv5(key))) @ wkl.T + bkl
       = conv5tap(key, W5c) + bkl_eff

with W5c[o,i,d] = A5[o,i,d] + A3[o,i,d-1] (A* = wkl-slice @ (bn_scale * conv_w*)).
This makes every tensor needed by head h a single-stage (shifted) matmul of the
raw inputs, so work shards cleanly over (batch, head-group) with no
collectives: core c handles batch c//2, heads 4*(c%2) .. 4*(c%2)+4.  Each core
emits the partial output projection of its 4 heads; the host adds the two
partials per batch plus the folded bias.

On-chip layout is feature-major ([d, L]).  Scores are computed transposed
([Lk, Lq]) so the AV matmul needs no transposes; a ones-column appended to V
makes the softmax denominator fall out of the same PSUM accumulation
(row 64), normalized via reciprocal + gpsimd partition_broadcast.

The q/kl/kg score operands are stored as fp8e4 so the score matmuls run in
DoubleRow perf mode (half cost per streamed column); the second DoubleRow
k-tile is a persistent zero slice, so the math is the plain 64-deep
contraction.  The value path (v, exp tiles, AV, output projection) stays
bf16 for accuracy.

The exp stream on the Activation engine is the critical resource, so the
emission is a software pipeline: score+exp tiles are emitted at the ACT
drain rate, and all other PE work (projections, the 5-tap conv, AV
accumulation, output projection) is emitted from a strict-FIFO queue of
filler generators between exp tiles, a few matmuls at a time.  Strict FIFO
keeps the 4-buffer PSUM work pool conflict-free (a generator finishes
before the next starts).
"""

import math
from contextlib import ExitStack

import ml_dtypes
import numpy as np

import concourse.tile as tile
from concourse import bacc, mybir
from concourse import bass_utils

F32 = mybir.dt.float32
BF16 = mybir.dt.bfloat16
FP8 = mybir.dt.float8e4
BF16_NP = ml_dtypes.bfloat16
DR = mybir.MatmulPerfMode.DoubleRow

B, L, D = 4, 2048, 512
H, DK = 8, 64
N_CORES = 8
HG = 4              # heads per core
DO = HG * DK        # 256 output dims per core
BN_EPS = 1e-5
NJ = D // 128       # 4 input-dim tiles
NLT = L // 128      # 16 L tiles of 128
NLQ = L // 512      # 4 lq blocks of 512

ET_BUFS = 27
PUMP = 2000         # filler PE cycles emitted per exp tile

_cache = {}


def _build_program(repeat=1, stages='all'):
    """Build + compile the per-core Bass program (same program on all cores)."""
    nc = bacc.Bacc("TRN2", target_bir_lowering=False, debug=False,
                   num_devices=N_CORES)

    dt_in = {}
    dt_in["xq"] = nc.dram_tensor("xq", [D, L], BF16, kind="ExternalInput").ap()
    dt_in["xk"] = nc.dram_tensor("xk", [D, L], BF16, kind="ExternalInput").ap()
    dt_in["xv"] = nc.dram_tensor("xv", [D, L], BF16, kind="ExternalInput").ap()
    dt_in["wq"] = nc.dram_tensor("wq", [D, DO], BF16, kind="ExternalInput").ap()
    dt_in["wk5"] = nc.dram_tensor("wk5", [5, D, DO], BF16, kind="ExternalInput").ap()
    dt_in["wkg"] = nc.dram_tensor("wkg", [D, DO], BF16, kind="ExternalInput").ap()
    dt_in["wv"] = nc.dram_tensor("wv", [D, DO], BF16, kind="ExternalInput").ap()
    dt_in["wo"] = nc.dram_tensor("wo", [64, HG, D], BF16, kind="ExternalInput").ap()
    dt_in["bkl"] = nc.dram_tensor("bkl", [DO], F32, kind="ExternalInput").ap()
    out_ap = nc.dram_tensor("out", [L, D], F32, kind="ExternalOutput").ap()

    with tile.TileContext(nc) as tc, ExitStack() as ctx:
        big = ctx.enter_context(tc.tile_pool(name="big", bufs=6))
        et = ctx.enter_context(tc.tile_pool(name="et", bufs=ET_BUFS))
        proj = ctx.enter_context(tc.tile_pool(name="projsb", bufs=1))
        norm = ctx.enter_context(tc.tile_pool(name="norm", bufs=2))
        ostage = ctx.enter_context(tc.tile_pool(name="ostage", bufs=3))
        sp = ctx.enter_context(tc.tile_pool(name="sp", bufs=2, space="PSUM"))
        work = ctx.enter_context(tc.tile_pool(name="work", bufs=4, space="PSUM"))

        # ---- persistent SBUF tensors (single-buffer pools) ----
        wq_sb = proj.tile([128, NJ, DO], BF16, tag="wq")
        wk5_sb = proj.tile([128, 5, NJ, DO], BF16, tag="wk5")
        wkg_sb = proj.tile([128, NJ, DO], BF16, tag="wkg")
        wv_sb = proj.tile([128, NJ, DO], BF16, tag="wv")
        wo_sb = proj.tile([64, HG, D], BF16, tag="wo")
        bkl_sb = proj.tile([128, 2], F32, tag="bkl")
        # fp8 score operands: [128, pair, i, L]; i=1 is a persistent zero
        # slice so the DoubleRow second k-tile contributes nothing.
        qT_sb = proj.tile([128, 2, 2, L], FP8, tag="qT")
        klT_sb = proj.tile([128, 2, 2, L], FP8, tag="klT")
        kgT_sb = proj.tile([128, 2, 2, L], FP8, tag="kgT")
        v_sb = proj.tile([128, NLT, HG, DK + 1], BF16, tag="v")
        xT_sb = [proj.tile([64, HG, L], BF16, tag=f"xT{br}", name=f"xT{br}")
                 for br in range(2)]
        dumm = proj.tile([1, 512], BF16, tag="dumm")

        # zero the DoubleRow i=1 slices on the (idle) gpsimd engine; p=0
        # slices first since phase 0 reads them earliest.
        for p in range(2):
            nc.gpsimd.memset(kgT_sb[:, p, 1, :], 0.0)
            nc.gpsimd.memset(qT_sb[:, p, 1, :], 0.0)
            nc.gpsimd.memset(klT_sb[:, p, 1, :], 0.0)
        nc.gpsimd.memset(v_sb[:], 1.0)

        # warm the ACT exp table and ramp the PE p-state with dummy matmuls
        warm = proj.tile([1, 16], F32, tag="warm")
        nc.vector.memset(warm[:], 0.0)
        nc.vector.memset(dumm[:], 0.0)
        nc.scalar.activation(warm[:], warm[:], mybir.ActivationFunctionType.Exp)

        def emit_body():
            # ---- load activations (feature-major), key padded for the conv;
            # j-pairs per transfer, spread across the four DMA queues ----
            LKP = L + 4  # padded length
            kx01 = big.tile([128, 2, LKP], BF16, tag="big", name="kx01")
            kx23 = big.tile([128, 2, LKP], BF16, tag="big", name="kx23")
            xq01 = big.tile([128, 2, LKP], BF16, tag="big", name="xq01")
            xq23 = big.tile([128, 2, LKP], BF16, tag="big", name="xq23")
            xv01 = big.tile([128, 2, LKP], BF16, tag="big", name="xv01")
            xv23 = big.tile([128, 2, LKP], BF16, tag="big", name="xv23")
            for t in (kx01, kx23):
                nc.vector.memset(t[:, :, 0:2], 0.0)
                nc.vector.memset(t[:, :, 2 + L:], 0.0)
            nc.sync.dma_start(kx01[:, :, 2:2 + L],
                              dt_in["xk"][0:256, :].rearrange("(t p) l -> p t l", p=128))
            nc.gpsimd.dma_start(kx23[:, :, 2:2 + L],
                                dt_in["xk"][256:512, :].rearrange("(t p) l -> p t l", p=128))
            nc.scalar.dma_start(wq_sb[:], dt_in["wq"].rearrange("(j p) o -> p j o", p=128))
            nc.scalar.dma_start(wkg_sb[:], dt_in["wkg"].rearrange("(j p) o -> p j o", p=128))
            nc.scalar.dma_start(xq01[:, 0, :L], dt_in["xq"][0:128, :])
            nc.scalar.dma_start(xq01[:, 1, :L], dt_in["xq"][128:256, :])
            nc.sync.dma_start(xq23[:, 0, :L], dt_in["xq"][256:384, :])
            nc.gpsimd.dma_start(xq23[:, 1, :L], dt_in["xq"][384:512, :])
            for t5 in range(5):
                nc.gpsimd.dma_start(
                    wk5_sb[:, t5, :, :],
                    dt_in["wk5"][t5].rearrange("(j p) o -> p j o", p=128))
            nc.scalar.dma_start(bkl_sb[:], dt_in["bkl"].rearrange("(m p) -> p m", p=128))
            nc.scalar.dma_start(wv_sb[:], dt_in["wv"].rearrange("(j p) o -> p j o", p=128))
            nc.scalar.dma_start(wo_sb[:], dt_in["wo"])
            nc.sync.dma_start(xv01[:, :, :L],
                              dt_in["xv"][0:256, :].rearrange("(t p) l -> p t l", p=128))
            nc.sync.dma_start(xv23[:, :, :L],
                              dt_in["xv"][256:512, :].rearrange("(t p) l -> p t l", p=128))
            kx = [kx01[:, 0, :], kx01[:, 1, :], kx23[:, 0, :], kx23[:, 1, :]]
            xq = [xq01[:, 0, :], xq01[:, 1, :], xq23[:, 0, :], xq23[:, 1, :]]
            xv = [xv01[:, 0, :], xv01[:, 1, :], xv23[:, 0, :], xv23[:, 1, :]]

            # PE p-state ramp: a few throwaway matmuls while DMAs land
            dps = work.tile([1, 512], F32, tag="wk", name="dps")
            for _ in range(6):
                nc.tensor.matmul(dps[:], dumm[0:1, 0:1], dumm[:],
                                 start=True, stop=True)

            # ---- filler queue ----
            fillers = []

            def pump(budget):
                while budget > 0 and fillers:
                    try:
                        budget -= next(fillers[0])
                    except StopIteration:
                        fillers.pop(0)

            def drain():
                pump(float('inf'))

            # ---- filler generators (PE work between exp tiles) ----
            def g_proj(dst_sb, w_sb, m, qbs, src, bias=None, off=0):
                for qb in qbs:
                    ps = work.tile([128, 512], F32, tag="wk")
                    for j in range(NJ):
                        nc.tensor.matmul(ps[:], w_sb[:, j, m * 128:(m + 1) * 128],
                                         src[j][:, off + qb * 512:off + qb * 512 + 512],
                                         start=(j == 0), stop=(j == NJ - 1))
                        yield 512
                    if bias is not None:
                        nc.vector.tensor_scalar_add(
                            dst_sb[:, m, 0, qb * 512:qb * 512 + 512], ps[:],
                            bias[:, m:m + 1])
                    else:
                        nc.vector.tensor_copy(
                            dst_sb[:, m, 0, qb * 512:qb * 512 + 512], ps[:])
                    yield 0

            def g_klT(m, qbs):
                # folded 5-tap conv projection: shifted slices of padded key
                for qb in qbs:
                    ps = work.tile([128, 512], F32, tag="wk")
                    first = True
                    for t in range(5):
                        for j in range(NJ):
                            sh = qb * 512 + t  # (t-2) shift + 2 pad offset
                            nc.tensor.matmul(
                                ps[:], wk5_sb[:, t, j, m * 128:(m + 1) * 128],
                                kx[j][:, sh:sh + 512],
                                start=first, stop=(t == 4 and j == NJ - 1))
                            first = False
                            yield 512
                    nc.vector.tensor_scalar_add(
                        klT_sb[:, m, 0, qb * 512:qb * 512 + 512], ps[:],
                        bkl_sb[:, m:m + 1])
                    yield 0

            def g_v(lts):
                for lt in lts:
                    ps = work.tile([128, 512], F32, tag="wk")
                    for j in range(NJ):
                        nc.tensor.matmul(ps[:, :DO], xv[j][:, lt * 128:lt * 128 + 128],
                                         wv_sb[:, j, :],
                                         start=(j == 0), stop=(j == NJ - 1))
                        yield 256
                    nc.vector.tensor_copy(
                        v_sb[:, lt, :, 0:DK],
                        ps[:, :DO].rearrange("p (h d) -> p h d", h=HG))
                    yield 0

            def norm_one(br, h, c, av):
                rd = norm.tile([DK + 1, 512], F32, tag="rd")
                nc.vector.reciprocal(rd[DK:DK + 1, :], av[DK:DK + 1, :])
                # HW partition_broadcast reads absolute partition 0:
                # DMA-remap row 64 -> 0 first.
                r0 = norm.tile([1, 512], F32, tag="r0")
                nc.sync.dma_start(r0[:], rd[DK:DK + 1, :])
                bc = norm.tile([DK, 512], F32, tag="bc")
                nc.gpsimd.partition_broadcast(bc[:], r0[0:1, :])
                nc.vector.tensor_tensor(
                    xT_sb[br][:, h, c * 512:c * 512 + 512],
                    av[0:DK, :], bc[:], mybir.AluOpType.mult)

            def g_av(p, br, qh, eT):
                # both heads x both chunks accumulate concurrently
                # (4 psum accumulators), so eT tiles release along lk
                avs = [[work.tile([DK + 1, 512], F32, tag="wk",
                                  name=f"av{hh}_{i}") for i in range(2)]
                       for hh in range(2)]
                for lk in range(NLT):
                    for hh in range(2):
                        for i in range(2):
                            nc.tensor.matmul(
                                avs[hh][i][:], v_sb[:, lk, 2 * p + hh, :],
                                eT[(lk, 2 * qh + i)][:, hh * 512:hh * 512 + 512],
                                start=(lk == 0), stop=(lk == NLT - 1))
                            yield 512
                for hh in range(2):
                    for i in range(2):
                        norm_one(br, 2 * p + hh, 2 * qh + i, avs[hh][i])
                        yield 0

            def g_av2(p, br, c, eT):
                # 2-accumulator variant for the tail chunks
                avs = [work.tile([DK + 1, 512], F32, tag="wk",
                                 name=f"avc{c}{hh}") for hh in range(2)]
                for lk in range(NLT):
                    for hh in range(2):
                        nc.tensor.matmul(
                            avs[hh][:], v_sb[:, lk, 2 * p + hh, :],
                            eT[(lk, c)][:, hh * 512:hh * 512 + 512],
                            start=(lk == 0), stop=(lk == NLT - 1))
                        yield 512
                for hh in range(2):
                    norm_one(br, 2 * p + hh, c, avs[hh])
                    yield 0

            def g_outproj(lts):
                for lt in lts:
                    po = work.tile([128, 512], F32, tag="wk")
                    k = 0
                    for br in range(2):
                        for h in range(HG):
                            nc.tensor.matmul(
                                po[:], xT_sb[br][:, h, lt * 128:lt * 128 + 128],
                                wo_sb[:, h, :],
                                start=(k == 0), stop=(k == 2 * HG - 1))
                            k += 1
                            yield 512
                    ot = ostage.tile([128, D], F32, tag="ot")
                    nc.vector.tensor_copy(ot[:], po[:])
                    nc.sync.dma_start(out_ap[lt * 128:lt * 128 + 128, :], ot[:])
                    yield 0

            # ---- score + exp driver, paced against the filler queue ----
            def phase_scores(p, br, lks=None, cs=(), eT=None):
                kT = klT_sb if br == 0 else kgT_sb
                if eT is None:
                    eT = {}
                for lk in (range(NLT) if lks is None else lks):
                    for c in cs:
                        ps = sp.tile([128, 1024], F32, tag="sp")
                        for hh in range(2):
                            pb = 64 * hh
                            nc.tensor.matmul(
                                ps[:, hh * 512:hh * 512 + 512],
                                kT[pb:pb + 64, p, :, lk * 128:lk * 128 + 128],
                                qT_sb[pb:pb + 64, p, :, c * 512:c * 512 + 512],
                                start=True, stop=True, perf_mode=DR)
                        e_t = et.tile([128, 1024], BF16, tag="et")
                        nc.scalar.activation(e_t[:], ps[:],
                                             mybir.ActivationFunctionType.Exp)
                        eT[(lk, c)] = e_t
                        pump(PUMP)
                return eT

            # ---- prologue: the projections phase 0 needs, emitted directly
            def run_now(g):
                for _ in g:
                    pass

            run_now(g_proj(kgT_sb, wkg_sb, 0, [0], kx, off=2))
            run_now(g_proj(qT_sb, wq_sb, 0, [0, 1], xq))
            if stages == 'proj':
                run_now(g_proj(qT_sb, wq_sb, 0, [2, 3], xq))
                run_now(g_klT(0, range(NLQ)))
                run_now(g_proj(kgT_sb, wkg_sb, 0, [1, 2, 3], kx, off=2))
                run_now(g_v(range(NLT)))
                run_now(g_proj(qT_sb, wq_sb, 1, range(NLQ), xq))
                run_now(g_klT(1, range(NLQ)))
                run_now(g_proj(kgT_sb, wkg_sb, 1, range(NLQ), kx, off=2))
                return
            do_av = stages != 'scores'

            # ---- pipelined phases: (pair, branch) x lq-half ----
            e = [None] * 8
            fillers.append(g_proj(kgT_sb, wkg_sb, 0, [1, 2, 3], kx, off=2))
            fillers.append(g_proj(kgT_sb, wkg_sb, 1, [0, 1, 2, 3], kx, off=2))
            fillers.append(g_proj(qT_sb, wq_sb, 1, [0, 1], xq))
            fillers.append(g_klT(0, [0, 1, 2, 3]))
            e[0] = phase_scores(0, 1, cs=(0, 1))

            fillers.append(g_v(range(NLT)))
            if do_av:
                fillers.append(g_av(0, 1, 0, e[0]))
            e[1] = phase_scores(0, 0, cs=(0, 1))

            fillers.append(g_klT(1, [0, 1]))
            if do_av:
                fillers.append(g_av(0, 0, 0, e[1]))
            e[2] = phase_scores(1, 1, cs=(0, 1))

            fillers.append(g_klT(1, [2, 3]))
            if do_av:
                fillers.append(g_av(1, 1, 0, e[2]))
            fillers.append(g_proj(qT_sb, wq_sb, 0, [2, 3], xq))
            fillers.append(g_proj(qT_sb, wq_sb, 1, [2, 3], xq))
            e[3] = phase_scores(1, 0, cs=(0, 1))

            if do_av:
                fillers.append(g_av(1, 0, 0, e[3]))
                fillers.append(g_outproj(range(0, 8)))
            e[4] = phase_scores(0, 1, cs=(2, 3))

            if do_av:
                fillers.append(g_av(0, 1, 1, e[4]))
            e[5] = phase_scores(0, 0, cs=(2, 3))

            if do_av:
                fillers.append(g_av(0, 0, 1, e[5]))
            e[6] = phase_scores(1, 1, cs=(2, 3))

            if do_av:
                fillers.append(g_av(1, 1, 1, e[6]))
            e[7] = phase_scores(1, 0, cs=(2,))

            if do_av:
                fillers.append(g_av2(1, 0, 2, e[7]))
                fillers.append(g_outproj(range(8, 12)))
            phase_scores(1, 0, cs=(3,), eT=e[7])

            if do_av:
                fillers.append(g_av2(1, 0, 3, e[7]))
                fillers.append(g_outproj(range(12, 16)))
            drain()

        for _rep in range(repeat):
            emit_body()

    nc.compile()
    return nc


def _host_prep(inputs):
    """Fold conv+bn+biases; build the 8 per-core input maps."""
    f32 = np.float32
    q = np.ascontiguousarray(inputs["query"], dtype=f32)
    k = np.ascontiguousarray(inputs["key"], dtype=f32)
    v = np.ascontiguousarray(inputs["value"], dtype=f32)
    w3 = np.asarray(inputs["conv_w3"], f32)
    w5 = np.asarray(inputs["conv_w5"], f32)
    b3 = np.asarray(inputs["conv_b3"], f32)
    b5 = np.asarray(inputs["conv_b5"], f32)
    gam = np.asarray(inputs["bn_gamma"], f32)
    bet = np.asarray(inputs["bn_beta"], f32)
    mu = np.asarray(inputs["bn_mean"], f32)
    var = np.asarray(inputs["bn_var"], f32)
    wq = np.asarray(inputs["wq"], f32)
    bq = np.asarray(inputs["bq"], f32)
    wkl = np.asarray(inputs["wkl"], f32)
    bkl = np.asarray(inputs["bkl"], f32)
    wkg = np.asarray(inputs["wkg"], f32)
    bkg = np.asarray(inputs["bkg"], f32)
    wv = np.asarray(inputs["wv"], f32)
    bv = np.asarray(inputs["bv"], f32)
    wo = np.asarray(inputs["wo"], f32)
    bo = np.asarray(inputs["bo"], f32)

    # biases that would change the math in ways we don't model on-chip
    assert not np.any(bq) and not np.any(bkg), "nonzero q/kg bias unsupported"

    s_bn = gam / np.sqrt(var + BN_EPS)                       # [1024]
    shift = np.concatenate([b3, b5]) * s_bn + (bet - mu * s_bn)
    wkl_s = wkl * s_bn[None, :]                              # [512, 1024]
    A3 = np.einsum("oc,cit->oit", wkl_s[:, :D], w3)          # [512, 512, 3]
    A5 = np.einsum("oc,cit->oit", wkl_s[:, D:], w5)          # [512, 512, 5]
    W5c = A5.copy()
    W5c[:, :, 1:4] += A3
    bkl_eff = wkl @ shift + bkl                              # [512]
    wq_eff = wq / math.sqrt(DK)
    bo_eff = bo + wo @ (2.0 * bv)

    bf = BF16_NP
    in_maps = []
    for c in range(N_CORES):
        b = c // 2
        hg = c % 2
        sel = slice(hg * DO, hg * DO + DO)
        in_maps.append({
            "xq": np.ascontiguousarray(q[b].T).astype(bf),
            "xk": np.ascontiguousarray(k[b].T).astype(bf),
            "xv": np.ascontiguousarray(v[b].T).astype(bf),
            "wq": np.ascontiguousarray(wq_eff.T[:, sel]).astype(bf),
            "wk5": np.ascontiguousarray(W5c.transpose(2, 1, 0)[:, :, sel]).astype(bf),
            "wkg": np.ascontiguousarray(wkg.T[:, sel]).astype(bf),
            "wv": np.ascontiguousarray(wv.T[:, sel]).astype(bf),
            "wo": np.ascontiguousarray(wo.T[sel, :]).reshape(HG, DK, D)
                   .transpose(1, 0, 2).astype(bf).copy(),
            "bkl": np.ascontiguousarray(bkl_eff[sel]).astype(f32),
        })
    return in_maps, bo_eff


def kernel(**inputs) -> np.ndarray:
    if "nc" not in _cache:
        _cache["nc"] = _build_program()
    nc = _cache["nc"]
    in_maps, bo_eff = _host_prep(inputs)
    res = bass_utils.run_bass_kernel_spmd(
        nc, in_maps, core_ids=list(range(N_CORES)))
    out = np.zeros((B, L, D), np.float32)
    for c in range(N_CORES):
        out[c // 2] += res.results[c]["out"]
    out += bo_eff[None, None, :]
    return out


# revision 14
# speedup vs baseline: 1.0127x; 1.0127x over previous
"""Trainium2 Bass kernel for nn_MultiHeadedAttention_4269197492266.

Dual-branch multi-head attention where the "local" key path is a multi-scale
conv (k=3,5) + batchnorm + projection.  Host-side algebra folds the whole
local path into a single 5-tap convolution:

    kl = bn(concat(conv3(key), conv5(key))) @ wkl.T + bkl
       = conv5tap(key, W5c) + bkl_eff

with W5c[o,i,d] = A5[o,i,d] + A3[o,i,d-1] (A* = wkl-slice @ (bn_scale * conv_w*)).
This makes every tensor needed by head h a single-stage (shifted) matmul of the
raw inputs, so work shards cleanly over (batch, head-group) with no
collectives: core c handles batch c//2, heads 4*(c%2) .. 4*(c%2)+4.  Each core
emits the partial output projection of its 4 heads; the host adds the two
partials per batch plus the folded bias.

On-chip layout is feature-major ([d, L]).  Scores are computed transposed
([Lk, Lq]) so the AV matmul needs no transposes; a ones-column appended to V
makes the softmax denominator fall out of the same PSUM accumulation
(row 64), normalized via reciprocal + gpsimd partition_broadcast.

The q/kl/kg score operands are stored as fp8e4 so the score matmuls run in
DoubleRow perf mode (half cost per streamed column); the second DoubleRow
k-tile is a persistent zero slice, so the math is the plain 64-deep
contraction.  The value path (v, exp tiles, AV, output projection) stays
bf16 for accuracy.

The exp stream on the Activation engine is the critical resource, so the
emission is a software pipeline: score+exp tiles are emitted at the ACT
drain rate, and all other PE work (projections, the 5-tap conv, AV
accumulation, output projection) is emitted from a strict-FIFO queue of
filler generators between exp tiles, a few matmuls at a time.  Strict FIFO
keeps the 4-buffer PSUM work pool conflict-free (a generator finishes
before the next starts).
"""

import math
from contextlib import ExitStack

import ml_dtypes
import numpy as np

import concourse.tile as tile
from concourse import bacc, mybir
from concourse import bass_utils

F32 = mybir.dt.float32
BF16 = mybir.dt.bfloat16
FP8 = mybir.dt.float8e4
BF16_NP = ml_dtypes.bfloat16
DR = mybir.MatmulPerfMode.DoubleRow

B, L, D = 4, 2048, 512
H, DK = 8, 64
N_CORES = 8
HG = 4              # heads per core
DO = HG * DK        # 256 output dims per core
BN_EPS = 1e-5
NJ = D // 128       # 4 input-dim tiles
NLT = L // 128      # 16 L tiles of 128
NLQ = L // 512      # 4 lq blocks of 512

ET_BUFS = 31
PUMP = 2000         # filler PE cycles emitted per exp tile

_cache = {}


def _build_program(repeat=1, stages='all'):
    """Build + compile the per-core Bass program (same program on all cores)."""
    nc = bacc.Bacc("TRN2", target_bir_lowering=False, debug=False,
                   num_devices=N_CORES)

    dt_in = {}
    dt_in["xq"] = nc.dram_tensor("xq", [D, L], BF16, kind="ExternalInput").ap()
    dt_in["xk"] = nc.dram_tensor("xk", [D, L], BF16, kind="ExternalInput").ap()
    dt_in["xv"] = nc.dram_tensor("xv", [D, L], BF16, kind="ExternalInput").ap()
    dt_in["wq"] = nc.dram_tensor("wq", [D, DO], BF16, kind="ExternalInput").ap()
    dt_in["wk5"] = nc.dram_tensor("wk5", [5, D, DO], BF16, kind="ExternalInput").ap()
    dt_in["wkg"] = nc.dram_tensor("wkg", [D, DO], BF16, kind="ExternalInput").ap()
    dt_in["wv"] = nc.dram_tensor("wv", [D, DO], BF16, kind="ExternalInput").ap()
    dt_in["wo"] = nc.dram_tensor("wo", [64, HG, D], BF16, kind="ExternalInput").ap()
    dt_in["bkl"] = nc.dram_tensor("bkl", [DO], F32, kind="ExternalInput").ap()
    out_ap = nc.dram_tensor("out", [L, D], F32, kind="ExternalOutput").ap()

    with tile.TileContext(nc) as tc, ExitStack() as ctx:
        big = ctx.enter_context(tc.tile_pool(name="big", bufs=6))
        et = ctx.enter_context(tc.tile_pool(name="et", bufs=ET_BUFS))
        proj = ctx.enter_context(tc.tile_pool(name="projsb", bufs=1))
        norm = ctx.enter_context(tc.tile_pool(name="norm", bufs=2))
        ostage = ctx.enter_context(tc.tile_pool(name="ostage", bufs=2))
        sp = ctx.enter_context(tc.tile_pool(name="sp", bufs=2, space="PSUM"))
        work = ctx.enter_context(tc.tile_pool(name="work", bufs=4, space="PSUM"))

        # ---- persistent SBUF tensors (single-buffer pools) ----
        wq_sb = proj.tile([128, NJ, DO], BF16, tag="wq")
        wk5_sb = proj.tile([128, 5, NJ, DO], BF16, tag="wk5")
        wkg_sb = proj.tile([128, NJ, DO], BF16, tag="wkg")
        wv_sb = proj.tile([128, NJ, DO], BF16, tag="wv")
        wo_sb = proj.tile([64, HG, D], BF16, tag="wo")
        bkl_sb = proj.tile([128, 2], F32, tag="bkl")
        # fp8 score operands: [128, pair, i, L]; i=1 is a persistent zero
        # slice so the DoubleRow second k-tile contributes nothing.
        qT_sb = proj.tile([128, 2, 2, L], FP8, tag="qT")
        klT_sb = proj.tile([128, 2, 2, L], FP8, tag="klT")
        kgT_sb = proj.tile([128, 2, 2, L], FP8, tag="kgT")
        v_sb = proj.tile([128, NLT, HG, DK + 1], BF16, tag="v")
        xT_sb = [proj.tile([64, HG, L], BF16, tag=f"xT{br}", name=f"xT{br}")
                 for br in range(2)]
        dumm = proj.tile([1, 512], BF16, tag="dumm")

        # warm the ACT exp table and ramp the PE p-state with dummy matmuls
        warm = proj.tile([1, 16], F32, tag="warm")
        nc.vector.memset(warm[:], 0.0)
        nc.vector.memset(dumm[:], 0.0)
        nc.scalar.activation(warm[:], warm[:], mybir.ActivationFunctionType.Exp)

        def emit_body():
            # ---- load activations (feature-major), key padded for the conv;
            # j-pairs per transfer, spread across the four DMA queues ----
            LKP = L + 4  # padded length
            kx01 = big.tile([128, 2, LKP], BF16, tag="big", name="kx01")
            kx23 = big.tile([128, 2, LKP], BF16, tag="big", name="kx23")
            xq01 = big.tile([128, 2, LKP], BF16, tag="big", name="xq01")
            xq23 = big.tile([128, 2, LKP], BF16, tag="big", name="xq23")
            xv01 = big.tile([128, 2, LKP], BF16, tag="big", name="xv01")
            xv23 = big.tile([128, 2, LKP], BF16, tag="big", name="xv23")
            for t in (kx01, kx23):
                nc.vector.memset(t[:, :, 0:2], 0.0)
                nc.vector.memset(t[:, :, 2 + L:], 0.0)
            # gpsimd DMAs are software-DGE generated on the Pool engine, so
            # emit those triggers FIRST (before the Pool memsets below).
            nc.gpsimd.dma_start(kx23[:, :, 2:2 + L],
                                dt_in["xk"][256:512, :].rearrange("(t p) l -> p t l", p=128))
            nc.gpsimd.dma_start(xq23[:, 1, :L], dt_in["xq"][384:512, :])
            for t5 in range(5):
                nc.gpsimd.dma_start(
                    wk5_sb[:, t5, :, :],
                    dt_in["wk5"][t5].rearrange("(j p) o -> p j o", p=128))
            nc.gpsimd.dma_start(xv01[:, :, :L],
                                dt_in["xv"][0:256, :].rearrange("(t p) l -> p t l", p=128))
            nc.gpsimd.dma_start(xv23[:, :, :L],
                                dt_in["xv"][256:512, :].rearrange("(t p) l -> p t l", p=128))
            nc.sync.dma_start(kx01[:, :, 2:2 + L],
                              dt_in["xk"][0:256, :].rearrange("(t p) l -> p t l", p=128))
            nc.sync.dma_start(xq23[:, 0, :L], dt_in["xq"][256:384, :])
            nc.scalar.dma_start(wq_sb[:], dt_in["wq"].rearrange("(j p) o -> p j o", p=128))
            nc.scalar.dma_start(wkg_sb[:], dt_in["wkg"].rearrange("(j p) o -> p j o", p=128))
            nc.scalar.dma_start(xq01[:, 0, :L], dt_in["xq"][0:128, :])
            nc.scalar.dma_start(xq01[:, 1, :L], dt_in["xq"][128:256, :])
            nc.scalar.dma_start(bkl_sb[:], dt_in["bkl"].rearrange("(m p) -> p m", p=128))
            nc.scalar.dma_start(wv_sb[:], dt_in["wv"].rearrange("(j p) o -> p j o", p=128))
            nc.scalar.dma_start(wo_sb[:], dt_in["wo"])

            # zero the DoubleRow i=1 slices: p=0 halves on the (early-idle)
            # DVE in first-read order, p=1 halves + the V ones-columns on the
            # Pool engine behind its DMA generation work.
            nc.vector.memset(kgT_sb[:, 0, 1, :], 0.0)
            nc.vector.memset(qT_sb[:, 0, 1, :], 0.0)
            nc.vector.memset(klT_sb[:, 0, 1, :], 0.0)
            nc.gpsimd.memset(kgT_sb[:, 1, 1, :], 0.0)
            nc.gpsimd.memset(qT_sb[:, 1, 1, :], 0.0)
            nc.gpsimd.memset(klT_sb[:, 1, 1, :], 0.0)
            nc.gpsimd.memset(v_sb[:], 1.0)
            kx = [kx01[:, 0, :], kx01[:, 1, :], kx23[:, 0, :], kx23[:, 1, :]]
            xq = [xq01[:, 0, :], xq01[:, 1, :], xq23[:, 0, :], xq23[:, 1, :]]
            xv = [xv01[:, 0, :], xv01[:, 1, :], xv23[:, 0, :], xv23[:, 1, :]]

            # PE p-state ramp: a few throwaway matmuls while DMAs land
            dps = work.tile([1, 512], F32, tag="wk", name="dps")
            for _ in range(6):
                nc.tensor.matmul(dps[:], dumm[0:1, 0:1], dumm[:],
                                 start=True, stop=True)

            # ---- filler queue ----
            fillers = []

            def pump(budget):
                # None-yield = generator waiting on not-yet-emitted exps;
                # rotate it to the back (safe: only the guarded tail AV
                # yields None, and its PSUM accumulators are pre-allocated).
                spins = 0
                while budget > 0 and fillers and spins < len(fillers):
                    try:
                        c = next(fillers[0])
                    except StopIteration:
                        fillers.pop(0)
                        continue
                    if c is None:
                        fillers.append(fillers.pop(0))
                        spins += 1
                        continue
                    spins = 0
                    budget -= c

            def drain():
                pump(float('inf'))

            # ---- filler generators (PE work between exp tiles) ----
            def g_proj(dst_sb, w_sb, m, qbs, src, bias=None, off=0):
                for qb in qbs:
                    ps = work.tile([128, 512], F32, tag="wk")
                    for j in range(NJ):
                        nc.tensor.matmul(ps[:], w_sb[:, j, m * 128:(m + 1) * 128],
                                         src[j][:, off + qb * 512:off + qb * 512 + 512],
                                         start=(j == 0), stop=(j == NJ - 1))
                        yield 512
                    if bias is not None:
                        nc.vector.tensor_scalar_add(
                            dst_sb[:, m, 0, qb * 512:qb * 512 + 512], ps[:],
                            bias[:, m:m + 1])
                    else:
                        nc.vector.tensor_copy(
                            dst_sb[:, m, 0, qb * 512:qb * 512 + 512], ps[:])
                    yield 0

            def g_klT(m, qbs):
                # folded 5-tap conv projection: shifted slices of padded key
                for qb in qbs:
                    ps = work.tile([128, 512], F32, tag="wk")
                    first = True
                    for t in range(5):
                        for j in range(NJ):
                            sh = qb * 512 + t  # (t-2) shift + 2 pad offset
                            nc.tensor.matmul(
                                ps[:], wk5_sb[:, t, j, m * 128:(m + 1) * 128],
                                kx[j][:, sh:sh + 512],
                                start=first, stop=(t == 4 and j == NJ - 1))
                            first = False
                            yield 512
                    nc.vector.tensor_scalar_add(
                        klT_sb[:, m, 0, qb * 512:qb * 512 + 512], ps[:],
                        bkl_sb[:, m:m + 1])
                    yield 0

            def g_v(lts):
                for lt in lts:
                    ps = work.tile([128, 512], F32, tag="wk")
                    for j in range(NJ):
                        nc.tensor.matmul(ps[:, :DO], xv[j][:, lt * 128:lt * 128 + 128],
                                         wv_sb[:, j, :],
                                         start=(j == 0), stop=(j == NJ - 1))
                        yield 256
                    nc.vector.tensor_copy(
                        v_sb[:, lt, :, 0:DK],
                        ps[:, :DO].rearrange("p (h d) -> p h d", h=HG))
                    yield 0

            def norm_one(br, h, c, av):
                # one [65,512] tile: row 64 holds the reciprocal of the
                # softmax denominator, rows 0-63 its partition-broadcast.
                cb = norm.tile([DK + 1, 512], F32, tag="cb")
                nc.vector.reciprocal(cb[DK:DK + 1, :], av[DK:DK + 1, :])
                # HW partition_broadcast reads absolute partition 0:
                # DMA-remap row 64 -> 0 first.
                r0 = norm.tile([1, 512], F32, tag="r0")
                nc.sync.dma_start(r0[:], cb[DK:DK + 1, :])
                nc.gpsimd.partition_broadcast(cb[0:DK, :], r0[0:1, :])
                nc.vector.tensor_tensor(
                    xT_sb[br][:, h, c * 512:c * 512 + 512],
                    av[0:DK, :], cb[0:DK, :], mybir.AluOpType.mult)

            def g_av(p, br, qh, eT):
                # both heads x both chunks accumulate concurrently
                # (4 psum accumulators), so eT tiles release along lk
                avs = [[work.tile([DK + 1, 512], F32, tag="wk",
                                  name=f"av{hh}_{i}") for i in range(2)]
                       for hh in range(2)]
                for lk in range(NLT):
                    for hh in range(2):
                        for i in range(2):
                            nc.tensor.matmul(
                                avs[hh][i][:], v_sb[:, lk, 2 * p + hh, :],
                                eT[(lk, 2 * qh + i)][:, hh * 512:hh * 512 + 512],
                                start=(lk == 0), stop=(lk == NLT - 1))
                            yield 512
                for hh in range(2):
                    for i in range(2):
                        norm_one(br, 2 * p + hh, 2 * qh + i, avs[hh][i])
                        yield 0

            def g_av2(p, br, c, eT, hi=None):
                # 2-accumulator variant for the tail chunks; with `hi`, pace
                # the lk sweep behind the exp emission watermark hi[0].
                avs = [work.tile([DK + 1, 512], F32, tag="wk",
                                 name=f"avc{c}{hh}") for hh in range(2)]
                for lk in range(NLT):
                    while hi is not None and lk > hi[0]:
                        yield None
                    for hh in range(2):
                        nc.tensor.matmul(
                            avs[hh][:], v_sb[:, lk, 2 * p + hh, :],
                            eT[(lk, c)][:, hh * 512:hh * 512 + 512],
                            start=(lk == 0), stop=(lk == NLT - 1))
                        yield 512
                for hh in range(2):
                    norm_one(br, 2 * p + hh, c, avs[hh])
                    yield 0

            def g_outproj(lts):
                for lt in lts:
                    po = work.tile([128, 512], F32, tag="wk")
                    k = 0
                    for br in range(2):
                        for h in range(HG):
                            nc.tensor.matmul(
                                po[:], xT_sb[br][:, h, lt * 128:lt * 128 + 128],
                                wo_sb[:, h, :],
                                start=(k == 0), stop=(k == 2 * HG - 1))
                            k += 1
                            yield 512
                    ot = ostage.tile([128, D], F32, tag="ot")
                    nc.vector.tensor_copy(ot[:], po[:])
                    nc.sync.dma_start(out_ap[lt * 128:lt * 128 + 128, :], ot[:])
                    yield 0

            # ---- score + exp driver, paced against the filler queue ----
            def phase_scores(p, br, lks=None, cs=(), eT=None):
                kT = klT_sb if br == 0 else kgT_sb
                if eT is None:
                    eT = {}
                for lk in (range(NLT) if lks is None else lks):
                    for c in cs:
                        ps = sp.tile([128, 1024], F32, tag="sp")
                        for hh in range(2):
                            pb = 64 * hh
                            nc.tensor.matmul(
                                ps[:, hh * 512:hh * 512 + 512],
                                kT[pb:pb + 64, p, :, lk * 128:lk * 128 + 128],
                                qT_sb[pb:pb + 64, p, :, c * 512:c * 512 + 512],
                                start=True, stop=True, perf_mode=DR)
                        e_t = et.tile([128, 1024], BF16, tag="et")
                        nc.scalar.activation(e_t[:], ps[:],
                                             mybir.ActivationFunctionType.Exp)
                        eT[(lk, c)] = e_t
                        pump(PUMP)
                return eT

            # ---- prologue: the projections phase 0 needs, emitted directly
            def run_now(g):
                for _ in g:
                    pass

            run_now(g_proj(kgT_sb, wkg_sb, 0, [0], kx, off=2))
            run_now(g_proj(qT_sb, wq_sb, 0, [0, 1], xq))
            if stages == 'proj':
                run_now(g_proj(qT_sb, wq_sb, 0, [2, 3], xq))
                run_now(g_klT(0, range(NLQ)))
                run_now(g_proj(kgT_sb, wkg_sb, 0, [1, 2, 3], kx, off=2))
                run_now(g_v(range(NLT)))
                run_now(g_proj(qT_sb, wq_sb, 1, range(NLQ), xq))
                run_now(g_klT(1, range(NLQ)))
                run_now(g_proj(kgT_sb, wkg_sb, 1, range(NLQ), kx, off=2))
                return
            do_av = stages != 'scores'

            # ---- pipelined phases: (pair, branch) x lq-half ----
            e = [None] * 8
            fillers.append(g_proj(kgT_sb, wkg_sb, 0, [1, 2, 3], kx, off=2))
            fillers.append(g_proj(kgT_sb, wkg_sb, 1, [0, 1, 2, 3], kx, off=2))
            fillers.append(g_proj(qT_sb, wq_sb, 1, [0, 1], xq))
            fillers.append(g_klT(0, [0, 1, 2, 3]))
            e[0] = phase_scores(0, 1, cs=(0, 1))

            fillers.append(g_v(range(NLT)))
            if do_av:
                fillers.append(g_av(0, 1, 0, e[0]))
            e[1] = phase_scores(0, 0, cs=(0, 1))

            fillers.append(g_klT(1, [0, 1]))
            if do_av:
                fillers.append(g_av(0, 0, 0, e[1]))
            e[2] = phase_scores(1, 1, cs=(0, 1))

            fillers.append(g_klT(1, [2, 3]))
            if do_av:
                fillers.append(g_av(1, 1, 0, e[2]))
            fillers.append(g_proj(qT_sb, wq_sb, 0, [2, 3], xq))
            fillers.append(g_proj(qT_sb, wq_sb, 1, [2, 3], xq))
            e[3] = phase_scores(1, 0, cs=(0, 1))

            if do_av:
                fillers.append(g_av(1, 0, 0, e[3]))
                fillers.append(g_outproj(range(0, 8)))
            e[4] = phase_scores(0, 1, cs=(2, 3))

            if do_av:
                fillers.append(g_av(0, 1, 1, e[4]))
            e[5] = phase_scores(0, 0, cs=(2, 3))

            if do_av:
                fillers.append(g_av(0, 0, 1, e[5]))
            e[6] = phase_scores(1, 1, cs=(2, 3))

            if do_av:
                fillers.append(g_av(1, 1, 1, e[6]))
            e[7] = phase_scores(1, 0, cs=(2,))

            # final c3 sweep: the c2 AV drains first, then the c3 AV trails
            # the exp watermark, then the qh1 output projection.
            c3hi = [-1]
            if do_av:
                fillers.append(g_av2(1, 0, 2, e[7]))
                fillers.append(g_av2(1, 0, 3, e[7], hi=c3hi))
                fillers.append(g_outproj(range(8, 12)))
            for lk in range(NLT):
                phase_scores(1, 0, lks=[lk], cs=(3,), eT=e[7])
                c3hi[0] = lk
            if do_av:
                fillers.append(g_outproj(range(12, 16)))
            drain()

        for _rep in range(repeat):
            emit_body()

    nc.compile()
    return nc


def _host_prep(inputs):
    """Fold conv+bn+biases; build the 8 per-core input maps."""
    f32 = np.float32
    q = np.ascontiguousarray(inputs["query"], dtype=f32)
    k = np.ascontiguousarray(inputs["key"], dtype=f32)
    v = np.ascontiguousarray(inputs["value"], dtype=f32)
    w3 = np.asarray(inputs["conv_w3"], f32)
    w5 = np.asarray(inputs["conv_w5"], f32)
    b3 = np.asarray(inputs["conv_b3"], f32)
    b5 = np.asarray(inputs["conv_b5"], f32)
    gam = np.asarray(inputs["bn_gamma"], f32)
    bet = np.asarray(inputs["bn_beta"], f32)
    mu = np.asarray(inputs["bn_mean"], f32)
    var = np.asarray(inputs["bn_var"], f32)
    wq = np.asarray(inputs["wq"], f32)
    bq = np.asarray(inputs["bq"], f32)
    wkl = np.asarray(inputs["wkl"], f32)
    bkl = np.asarray(inputs["bkl"], f32)
    wkg = np.asarray(inputs["wkg"], f32)
    bkg = np.asarray(inputs["bkg"], f32)
    wv = np.asarray(inputs["wv"], f32)
    bv = np.asarray(inputs["bv"], f32)
    wo = np.asarray(inputs["wo"], f32)
    bo = np.asarray(inputs["bo"], f32)

    # biases that would change the math in ways we don't model on-chip
    assert not np.any(bq) and not np.any(bkg), "nonzero q/kg bias unsupported"

    s_bn = gam / np.sqrt(var + BN_EPS)                       # [1024]
    shift = np.concatenate([b3, b5]) * s_bn + (bet - mu * s_bn)
    wkl_s = wkl * s_bn[None, :]                              # [512, 1024]
    A3 = np.einsum("oc,cit->oit", wkl_s[:, :D], w3)          # [512, 512, 3]
    A5 = np.einsum("oc,cit->oit", wkl_s[:, D:], w5)          # [512, 512, 5]
    W5c = A5.copy()
    W5c[:, :, 1:4] += A3
    bkl_eff = wkl @ shift + bkl                              # [512]
    wq_eff = wq / math.sqrt(DK)
    bo_eff = bo + wo @ (2.0 * bv)

    bf = BF16_NP
    in_maps = []
    for c in range(N_CORES):
        b = c // 2
        hg = c % 2
        sel = slice(hg * DO, hg * DO + DO)
        in_maps.append({
            "xq": np.ascontiguousarray(q[b].T).astype(bf),
            "xk": np.ascontiguousarray(k[b].T).astype(bf),
            "xv": np.ascontiguousarray(v[b].T).astype(bf),
            "wq": np.ascontiguousarray(wq_eff.T[:, sel]).astype(bf),
            "wk5": np.ascontiguousarray(W5c.transpose(2, 1, 0)[:, :, sel]).astype(bf),
            "wkg": np.ascontiguousarray(wkg.T[:, sel]).astype(bf),
            "wv": np.ascontiguousarray(wv.T[:, sel]).astype(bf),
            "wo": np.ascontiguousarray(wo.T[sel, :]).reshape(HG, DK, D)
                   .transpose(1, 0, 2).astype(bf).copy(),
            "bkl": np.ascontiguousarray(bkl_eff[sel]).astype(f32),
        })
    return in_maps, bo_eff


def kernel(**inputs) -> np.ndarray:
    if "nc" not in _cache:
        _cache["nc"] = _build_program()
    nc = _cache["nc"]
    in_maps, bo_eff = _host_prep(inputs)
    res = bass_utils.run_bass_kernel_spmd(
        nc, in_maps, core_ids=list(range(N_CORES)))
    out = np.zeros((B, L, D), np.float32)
    for c in range(N_CORES):
        out[c // 2] += res.results[c]["out"]
    out += bo_eff[None, None, :]
    return out


# revision 15
# speedup vs baseline: 1.0250x; 1.0122x over previous
"""Trainium2 Bass kernel for nn_MultiHeadedAttention_4269197492266.

Dual-branch multi-head attention where the "local" key path is a multi-scale
conv (k=3,5) + batchnorm + projection.  Host-side algebra folds the whole
local path into a single 5-tap convolution:

    kl = bn(concat(conv3(key), conv5(key))) @ wkl.T + bkl
       = conv5tap(key, W5c) + bkl_eff

with W5c[o,i,d] = A5[o,i,d] + A3[o,i,d-1] (A* = wkl-slice @ (bn_scale * conv_w*)).
This makes every tensor needed by head h a single-stage (shifted) matmul of the
raw inputs, so work shards cleanly over (batch, head-group) with no
collectives: core c handles batch c//2, heads 4*(c%2) .. 4*(c%2)+4.  Each core
emits the partial output projection of its 4 heads; the host adds the two
partials per batch plus the folded bias.

On-chip layout is feature-major ([d, L]).  Scores are computed transposed
([Lk, Lq]) so the AV matmul needs no transposes; a ones-column appended to V
makes the softmax denominator fall out of the same PSUM accumulation
(row 64), normalized via reciprocal + gpsimd partition_broadcast.

The q/kl/kg score operands are stored as fp8e4 so the score matmuls run in
DoubleRow perf mode (half cost per streamed column); the second DoubleRow
k-tile is a persistent zero slice, so the math is the plain 64-deep
contraction.  The value path (v, exp tiles, AV, output projection) stays
bf16 for accuracy.

The exp stream on the Activation engine is the critical resource, so the
emission is a software pipeline: score+exp tiles are emitted at the ACT
drain rate, and all other PE work (projections, the 5-tap conv, AV
accumulation, output projection) is emitted from a strict-FIFO queue of
filler generators between exp tiles, a few matmuls at a time.  Strict FIFO
keeps the 4-buffer PSUM work pool conflict-free (a generator finishes
before the next starts).
"""

import math
from contextlib import ExitStack

import ml_dtypes
import numpy as np

import concourse.tile as tile
from concourse import bacc, mybir
from concourse import bass_utils

F32 = mybir.dt.float32
BF16 = mybir.dt.bfloat16
FP8 = mybir.dt.float8e4
BF16_NP = ml_dtypes.bfloat16
DR = mybir.MatmulPerfMode.DoubleRow

B, L, D = 4, 2048, 512
H, DK = 8, 64
N_CORES = 8
HG = 4              # heads per core
DO = HG * DK        # 256 output dims per core
BN_EPS = 1e-5
NJ = D // 128       # 4 input-dim tiles
NLT = L // 128      # 16 L tiles of 128
NLQ = L // 512      # 4 lq blocks of 512

ET_BUFS = 37
PUMP = 2000         # filler PE cycles emitted per exp tile

_cache = {}


def _build_program(repeat=1, stages='all'):
    """Build + compile the per-core Bass program (same program on all cores)."""
    nc = bacc.Bacc("TRN2", target_bir_lowering=False, debug=False,
                   num_devices=N_CORES)

    dt_in = {}
    dt_in["xq"] = nc.dram_tensor("xq", [D, L], BF16, kind="ExternalInput").ap()
    dt_in["xk"] = nc.dram_tensor("xk", [D, L], BF16, kind="ExternalInput").ap()
    dt_in["xv"] = nc.dram_tensor("xv", [D, L], BF16, kind="ExternalInput").ap()
    dt_in["wq"] = nc.dram_tensor("wq", [D, DO], BF16, kind="ExternalInput").ap()
    dt_in["wk5"] = nc.dram_tensor("wk5", [5, D, DO], BF16, kind="ExternalInput").ap()
    dt_in["wkg"] = nc.dram_tensor("wkg", [D, DO], BF16, kind="ExternalInput").ap()
    dt_in["wv"] = nc.dram_tensor("wv", [D, DO], BF16, kind="ExternalInput").ap()
    dt_in["wo"] = nc.dram_tensor("wo", [64, HG, D], BF16, kind="ExternalInput").ap()
    dt_in["bkl"] = nc.dram_tensor("bkl", [DO], F32, kind="ExternalInput").ap()
    out_ap = nc.dram_tensor("out", [L, D], F32, kind="ExternalOutput").ap()

    with tile.TileContext(nc) as tc, ExitStack() as ctx:
        big = ctx.enter_context(tc.tile_pool(name="big", bufs=6))
        et = ctx.enter_context(tc.tile_pool(name="et", bufs=ET_BUFS))
        proj = ctx.enter_context(tc.tile_pool(name="projsb", bufs=1))
        norm = ctx.enter_context(tc.tile_pool(name="norm", bufs=2))
        ostage = ctx.enter_context(tc.tile_pool(name="ostage", bufs=2))
        sp = ctx.enter_context(tc.tile_pool(name="sp", bufs=2, space="PSUM"))
        work = ctx.enter_context(tc.tile_pool(name="work", bufs=4, space="PSUM"))

        # ---- persistent SBUF tensors (single-buffer pools) ----
        wq_sb = proj.tile([128, NJ, DO], BF16, tag="wq")
        wk5_sb = proj.tile([128, 5, NJ, DO], BF16, tag="wk5")
        wkg_sb = proj.tile([128, NJ, DO], BF16, tag="wkg")
        wv_sb = proj.tile([128, NJ, DO], BF16, tag="wv")
        wo_sb = proj.tile([64, HG, D], BF16, tag="wo")
        bkl_sb = proj.tile([128, 2], F32, tag="bkl")
        # fp8 score operands [128, pair, L]; the DoubleRow i-dim is a
        # stride-0 broadcast (both k-tiles read the same data), compensated
        # by halving wq on the host.
        qT_sb = proj.tile([128, 2, L], FP8, tag="qT")
        klT_sb = proj.tile([128, 2, L], FP8, tag="klT")
        kgT_sb = proj.tile([128, 2, L], FP8, tag="kgT")
        v_sb = proj.tile([128, NLT, HG, DK + 1], BF16, tag="v")
        xT_sb = [proj.tile([64, HG, L], BF16, tag=f"xT{br}", name=f"xT{br}")
                 for br in range(2)]

        # warm the ACT exp table and ramp the PE p-state with dummy matmuls
        warm = proj.tile([1, 16], F32, tag="warm")
        nc.vector.memset(warm[:], 0.0)
        nc.scalar.activation(warm[:], warm[:], mybir.ActivationFunctionType.Exp)

        def emit_body():
            # ---- load activations (feature-major), key padded for the conv;
            # j-pairs per transfer, spread across the four DMA queues ----
            LKP = L + 4  # padded length
            kx01 = big.tile([128, 2, LKP], BF16, tag="big", name="kx01")
            kx23 = big.tile([128, 2, LKP], BF16, tag="big", name="kx23")
            xq01 = big.tile([128, 2, LKP], BF16, tag="big", name="xq01")
            xq23 = big.tile([128, 2, LKP], BF16, tag="big", name="xq23")
            xv01 = big.tile([128, 2, LKP], BF16, tag="big", name="xv01")
            xv23 = big.tile([128, 2, LKP], BF16, tag="big", name="xv23")
            for t in (kx01, kx23):
                nc.vector.memset(t[:, :, 0:2], 0.0)
                nc.vector.memset(t[:, :, 2 + L:], 0.0)
            # gpsimd DMAs are software-DGE generated on the Pool engine, so
            # emit those triggers FIRST (before the Pool memsets below).
            nc.gpsimd.dma_start(kx23[:, :, 2:2 + L],
                                dt_in["xk"][256:512, :].rearrange("(t p) l -> p t l", p=128))
            nc.gpsimd.dma_start(xq23[:, 1, :L], dt_in["xq"][384:512, :])
            for t5 in range(5):
                nc.gpsimd.dma_start(
                    wk5_sb[:, t5, :, :],
                    dt_in["wk5"][t5].rearrange("(j p) o -> p j o", p=128))
            nc.gpsimd.dma_start(xv01[:, :, :L],
                                dt_in["xv"][0:256, :].rearrange("(t p) l -> p t l", p=128))
            nc.gpsimd.dma_start(xv23[:, :, :L],
                                dt_in["xv"][256:512, :].rearrange("(t p) l -> p t l", p=128))
            nc.sync.dma_start(kx01[:, :, 2:2 + L],
                              dt_in["xk"][0:256, :].rearrange("(t p) l -> p t l", p=128))
            nc.sync.dma_start(xq23[:, 0, :L], dt_in["xq"][256:384, :])
            nc.scalar.dma_start(wq_sb[:], dt_in["wq"].rearrange("(j p) o -> p j o", p=128))
            nc.scalar.dma_start(wkg_sb[:], dt_in["wkg"].rearrange("(j p) o -> p j o", p=128))
            nc.scalar.dma_start(xq01[:, 0, :L], dt_in["xq"][0:128, :])
            nc.scalar.dma_start(xq01[:, 1, :L], dt_in["xq"][128:256, :])
            nc.scalar.dma_start(bkl_sb[:], dt_in["bkl"].rearrange("(m p) -> p m", p=128))
            nc.scalar.dma_start(wv_sb[:], dt_in["wv"].rearrange("(j p) o -> p j o", p=128))
            nc.scalar.dma_start(wo_sb[:], dt_in["wo"])

            # V ones-columns on the Pool engine behind its DMA generation
            nc.gpsimd.memset(v_sb[:], 1.0)
            kx = [kx01[:, 0, :], kx01[:, 1, :], kx23[:, 0, :], kx23[:, 1, :]]
            xq = [xq01[:, 0, :], xq01[:, 1, :], xq23[:, 0, :], xq23[:, 1, :]]
            xv = [xv01[:, 0, :], xv01[:, 1, :], xv23[:, 0, :], xv23[:, 1, :]]

            # ---- filler queue ----
            fillers = []

            def pump(budget):
                # None-yield = generator waiting on not-yet-emitted exps;
                # rotate it to the back (safe: only the guarded tail AV
                # yields None, and its PSUM accumulators are pre-allocated).
                spins = 0
                while budget > 0 and fillers and spins < len(fillers):
                    try:
                        c = next(fillers[0])
                    except StopIteration:
                        fillers.pop(0)
                        continue
                    if c is None:
                        fillers.append(fillers.pop(0))
                        spins += 1
                        continue
                    spins = 0
                    budget -= c

            def drain():
                pump(float('inf'))

            # ---- filler generators (PE work between exp tiles) ----
            def g_proj(dst_sb, w_sb, m, qbs, src, bias=None, off=0):
                for qb in qbs:
                    ps = work.tile([128, 512], F32, tag="wk")
                    for j in range(NJ):
                        nc.tensor.matmul(ps[:], w_sb[:, j, m * 128:(m + 1) * 128],
                                         src[j][:, off + qb * 512:off + qb * 512 + 512],
                                         start=(j == 0), stop=(j == NJ - 1))
                        yield 512
                    if bias is not None:
                        nc.vector.tensor_scalar_add(
                            dst_sb[:, m, qb * 512:qb * 512 + 512], ps[:],
                            bias[:, m:m + 1])
                    else:
                        nc.vector.tensor_copy(
                            dst_sb[:, m, qb * 512:qb * 512 + 512], ps[:])
                    yield 0

            def g_klT(m, qbs):
                # folded 5-tap conv projection: shifted slices of padded key
                for qb in qbs:
                    ps = work.tile([128, 512], F32, tag="wk")
                    first = True
                    for t in range(5):
                        for j in range(NJ):
                            sh = qb * 512 + t  # (t-2) shift + 2 pad offset
                            nc.tensor.matmul(
                                ps[:], wk5_sb[:, t, j, m * 128:(m + 1) * 128],
                                kx[j][:, sh:sh + 512],
                                start=first, stop=(t == 4 and j == NJ - 1))
                            first = False
                            yield 512
                    nc.vector.tensor_scalar_add(
                        klT_sb[:, m, qb * 512:qb * 512 + 512], ps[:],
                        bkl_sb[:, m:m + 1])
                    yield 0

            def g_v(lts):
                for lt in lts:
                    ps = work.tile([128, 512], F32, tag="wk")
                    for j in range(NJ):
                        nc.tensor.matmul(ps[:, :DO], xv[j][:, lt * 128:lt * 128 + 128],
                                         wv_sb[:, j, :],
                                         start=(j == 0), stop=(j == NJ - 1))
                        yield 256
                    nc.vector.tensor_copy(
                        v_sb[:, lt, :, 0:DK],
                        ps[:, :DO].rearrange("p (h d) -> p h d", h=HG))
                    yield 0

            def norm_one(br, h, c, av):
                # one [65,512] tile: row 64 holds the reciprocal of the
                # softmax denominator, rows 0-63 its partition-broadcast.
                cb = norm.tile([DK + 1, 512], F32, tag="cb")
                nc.vector.reciprocal(cb[DK:DK + 1, :], av[DK:DK + 1, :])
                # HW partition_broadcast reads absolute partition 0:
                # DMA-remap row 64 -> 0 first.
                r0 = norm.tile([1, 512], F32, tag="r0")
                nc.sync.dma_start(r0[:], cb[DK:DK + 1, :])
                nc.gpsimd.partition_broadcast(cb[0:DK, :], r0[0:1, :])
                nc.vector.tensor_tensor(
                    xT_sb[br][:, h, c * 512:c * 512 + 512],
                    av[0:DK, :], cb[0:DK, :], mybir.AluOpType.mult)

            def g_av(p, br, qh, eT):
                # both heads x both chunks accumulate concurrently
                # (4 psum accumulators), so eT tiles release along lk
                avs = [[work.tile([DK + 1, 512], F32, tag="wk",
                                  name=f"av{hh}_{i}") for i in range(2)]
                       for hh in range(2)]
                for lk in range(NLT):
                    for hh in range(2):
                        for i in range(2):
                            nc.tensor.matmul(
                                avs[hh][i][:], v_sb[:, lk, 2 * p + hh, :],
                                eT[(lk, 2 * qh + i)][:, hh * 512:hh * 512 + 512],
                                start=(lk == 0), stop=(lk == NLT - 1))
                            yield 512
                for hh in range(2):
                    for i in range(2):
                        norm_one(br, 2 * p + hh, 2 * qh + i, avs[hh][i])
                        yield 0

            def g_av2(p, br, c, eT, hi=None):
                # 2-accumulator variant for the tail chunks; with `hi`, pace
                # the lk sweep behind the exp emission watermark hi[0].
                avs = [work.tile([DK + 1, 512], F32, tag="wk",
                                 name=f"avc{c}{hh}") for hh in range(2)]
                for lk in range(NLT):
                    while hi is not None and lk > hi[0]:
                        yield None
                    for hh in range(2):
                        nc.tensor.matmul(
                            avs[hh][:], v_sb[:, lk, 2 * p + hh, :],
                            eT[(lk, c)][:, hh * 512:hh * 512 + 512],
                            start=(lk == 0), stop=(lk == NLT - 1))
                        yield 512
                for hh in range(2):
                    norm_one(br, 2 * p + hh, c, avs[hh])
                    yield 0

            def g_outproj(lts):
                for lt in lts:
                    po = work.tile([128, 512], F32, tag="wk")
                    k = 0
                    for br in range(2):
                        for h in range(HG):
                            nc.tensor.matmul(
                                po[:], xT_sb[br][:, h, lt * 128:lt * 128 + 128],
                                wo_sb[:, h, :],
                                start=(k == 0), stop=(k == 2 * HG - 1))
                            k += 1
                            yield 512
                    ot = ostage.tile([128, D], F32, tag="ot")
                    nc.vector.tensor_copy(ot[:], po[:])
                    nc.sync.dma_start(out_ap[lt * 128:lt * 128 + 128, :], ot[:])
                    yield 0

            # ---- score + exp driver, paced against the filler queue ----
            def phase_scores(p, br, lks=None, cs=(), eT=None):
                kT = klT_sb if br == 0 else kgT_sb
                if eT is None:
                    eT = {}
                for lk in (range(NLT) if lks is None else lks):
                    for c in cs:
                        ps = sp.tile([128, 1024], F32, tag="sp")
                        for hh in range(2):
                            pb = 64 * hh
                            nc.tensor.matmul(
                                ps[:, hh * 512:hh * 512 + 512],
                                kT[pb:pb + 64, p, lk * 128:lk * 128 + 128]
                                .unsqueeze(1).broadcast_to((64, 2, 128)),
                                qT_sb[pb:pb + 64, p, c * 512:c * 512 + 512]
                                .unsqueeze(1).broadcast_to((64, 2, 512)),
                                start=True, stop=True, perf_mode=DR)
                        e_t = et.tile([128, 1024], BF16, tag="et")
                        nc.scalar.activation(e_t[:], ps[:],
                                             mybir.ActivationFunctionType.Exp)
                        eT[(lk, c)] = e_t
                        pump(PUMP)
                return eT

            # ---- prologue: the projections phase 0 needs, emitted directly
            def run_now(g):
                for _ in g:
                    pass

            run_now(g_proj(kgT_sb, wkg_sb, 0, [0], kx, off=2))
            run_now(g_proj(qT_sb, wq_sb, 0, [0, 1], xq))
            if stages == 'proj':
                run_now(g_proj(qT_sb, wq_sb, 0, [2, 3], xq))
                run_now(g_klT(0, range(NLQ)))
                run_now(g_proj(kgT_sb, wkg_sb, 0, [1, 2, 3], kx, off=2))
                run_now(g_v(range(NLT)))
                run_now(g_proj(qT_sb, wq_sb, 1, range(NLQ), xq))
                run_now(g_klT(1, range(NLQ)))
                run_now(g_proj(kgT_sb, wkg_sb, 1, range(NLQ), kx, off=2))
                return
            do_av = stages != 'scores'

            # ---- pipelined phases: (pair, branch) x lq-half ----
            e = [None] * 8
            fillers.append(g_proj(kgT_sb, wkg_sb, 0, [1, 2, 3], kx, off=2))
            fillers.append(g_proj(kgT_sb, wkg_sb, 1, [0, 1, 2, 3], kx, off=2))
            fillers.append(g_proj(qT_sb, wq_sb, 1, [0, 1], xq))
            fillers.append(g_klT(0, [0, 1, 2, 3]))
            e[0] = phase_scores(0, 1, cs=(0, 1))

            fillers.append(g_v(range(NLT)))
            if do_av:
                fillers.append(g_av(0, 1, 0, e[0]))
            e[1] = phase_scores(0, 0, cs=(0, 1))

            fillers.append(g_klT(1, [0, 1]))
            if do_av:
                fillers.append(g_av(0, 0, 0, e[1]))
            e[2] = phase_scores(1, 1, cs=(0, 1))

            fillers.append(g_klT(1, [2, 3]))
            if do_av:
                fillers.append(g_av(1, 1, 0, e[2]))
            fillers.append(g_proj(qT_sb, wq_sb, 0, [2, 3], xq))
            fillers.append(g_proj(qT_sb, wq_sb, 1, [2, 3], xq))
            e[3] = phase_scores(1, 0, cs=(0, 1))

            if do_av:
                fillers.append(g_av(1, 0, 0, e[3]))
                fillers.append(g_outproj(range(0, 4)))
            e[4] = phase_scores(0, 1, cs=(2, 3))

            if do_av:
                fillers.append(g_av(0, 1, 1, e[4]))
                fillers.append(g_outproj(range(4, 8)))
            e[5] = phase_scores(0, 0, cs=(2, 3))

            if do_av:
                fillers.append(g_av(0, 0, 1, e[5]))
            e[6] = phase_scores(1, 1, cs=(2, 3))

            if do_av:
                fillers.append(g_av(1, 1, 1, e[6]))
            e[7] = phase_scores(1, 0, cs=(2,))

            # final c3 sweep: the c2 AV drains first, then the c3 AV trails
            # the exp watermark, then the qh1 output projection.
            c3hi = [-1]
            if do_av:
                fillers.append(g_av2(1, 0, 2, e[7]))
                fillers.append(g_av2(1, 0, 3, e[7], hi=c3hi))
                fillers.append(g_outproj(range(8, 12)))
            for lk in range(NLT):
                phase_scores(1, 0, lks=[lk], cs=(3,), eT=e[7])
                c3hi[0] = lk
            if do_av:
                fillers.append(g_outproj(range(12, 16)))
            drain()

        for _rep in range(repeat):
            emit_body()

    nc.compile()
    return nc


def _host_prep(inputs):
    """Fold conv+bn+biases; build the 8 per-core input maps."""
    f32 = np.float32
    q = np.ascontiguousarray(inputs["query"], dtype=f32)
    k = np.ascontiguousarray(inputs["key"], dtype=f32)
    v = np.ascontiguousarray(inputs["value"], dtype=f32)
    w3 = np.asarray(inputs["conv_w3"], f32)
    w5 = np.asarray(inputs["conv_w5"], f32)
    b3 = np.asarray(inputs["conv_b3"], f32)
    b5 = np.asarray(inputs["conv_b5"], f32)
    gam = np.asarray(inputs["bn_gamma"], f32)
    bet = np.asarray(inputs["bn_beta"], f32)
    mu = np.asarray(inputs["bn_mean"], f32)
    var = np.asarray(inputs["bn_var"], f32)
    wq = np.asarray(inputs["wq"], f32)
    bq = np.asarray(inputs["bq"], f32)
    wkl = np.asarray(inputs["wkl"], f32)
    bkl = np.asarray(inputs["bkl"], f32)
    wkg = np.asarray(inputs["wkg"], f32)
    bkg = np.asarray(inputs["bkg"], f32)
    wv = np.asarray(inputs["wv"], f32)
    bv = np.asarray(inputs["bv"], f32)
    wo = np.asarray(inputs["wo"], f32)
    bo = np.asarray(inputs["bo"], f32)

    # biases that would change the math in ways we don't model on-chip
    assert not np.any(bq) and not np.any(bkg), "nonzero q/kg bias unsupported"

    s_bn = gam / np.sqrt(var + BN_EPS)                       # [1024]
    shift = np.concatenate([b3, b5]) * s_bn + (bet - mu * s_bn)
    wkl_s = wkl * s_bn[None, :]                              # [512, 1024]
    A3 = np.einsum("oc,cit->oit", wkl_s[:, :D], w3)          # [512, 512, 3]
    A5 = np.einsum("oc,cit->oit", wkl_s[:, D:], w5)          # [512, 512, 5]
    W5c = A5.copy()
    W5c[:, :, 1:4] += A3
    bkl_eff = wkl @ shift + bkl                              # [512]
    wq_eff = wq * (0.5 / math.sqrt(DK))
    bo_eff = bo + wo @ (2.0 * bv)

    bf = BF16_NP
    in_maps = []
    for c in range(N_CORES):
        b = c // 2
        hg = c % 2
        sel = slice(hg * DO, hg * DO + DO)
        in_maps.append({
            "xq": np.ascontiguousarray(q[b].T).astype(bf),
            "xk": np.ascontiguousarray(k[b].T).astype(bf),
            "xv": np.ascontiguousarray(v[b].T).astype(bf),
            "wq": np.ascontiguousarray(wq_eff.T[:, sel]).astype(bf),
            "wk5": np.ascontiguousarray(W5c.transpose(2, 1, 0)[:, :, sel]).astype(bf),
            "wkg": np.ascontiguousarray(wkg.T[:, sel]).astype(bf),
            "wv": np.ascontiguousarray(wv.T[:, sel]).astype(bf),
            "wo": np.ascontiguousarray(wo.T[sel, :]).reshape(HG, DK, D)
                   .transpose(1, 0, 2).astype(bf).copy(),
            "bkl": np.ascontiguousarray(bkl_eff[sel]).astype(f32),
        })
    return in_maps, bo_eff


def kernel(**inputs) -> np.ndarray:
    if "nc" not in _cache:
        _cache["nc"] = _build_program()
    nc = _cache["nc"]
    in_maps, bo_eff = _host_prep(inputs)
    res = bass_utils.run_bass_kernel_spmd(
        nc, in_maps, core_ids=list(range(N_CORES)))
    out = np.zeros((B, L, D), np.float32)
    for c in range(N_CORES):
        out[c // 2] += res.results[c]["out"]
    out += bo_eff[None, None, :]
    return out
